# revision 8
# baseline (speedup 1.0000x reference)
"""Long convolution (FFT conv + residual) on 8 Trainium2 NeuronCores.

Math (identical to the reference):
  out[b,l,h] = x[b,l,h] + sum_{s<=l} x[b,s,h]*filt[h,l-s]
computed as a zero-padded circular convolution with an FFT of size
N = 2L = 8192. The residual is folded into the filter on the host
(filt[h,0] += 1), so the device computes only the convolution.

Device algorithm (per core, 128 of the 1024 channels — channel-sharded,
no inter-core communication):
  FFT(8192) is a four-step Cooley-Tukey factorization 8192 = 64 x 128
  mapped onto the TensorEngine as dense matmuls with bf16 inputs and
  fp32 PSUM accumulation:
    step1  B[k1,n2]   = sum_{n1<32} W64[n1,k1] * z[128*n1+n2]   (input
           is zero-padded: rows n1>=32 are zero, so K=32)
    tw     C = B * exp(-2pi i n2 k1 / 8192)     (VectorE, fp32 twiddles)
    T      C^T (per-sequence [64,128] PE transposes)
    step2  X[k2,(s,k1)] = sum_{n2} W128[n2,k2] * C^T[n2,(s,k1)]
  Two real sequences (batches 2p, 2p+1) are packed as one complex
  sequence z = x[2p] + i*x[2p+1]; since the filter is real, the real /
  imag parts of the inverse transform are the two convolutions.
  The filter's own FFT (real input, scaled by 1/8192) is computed on
  device once and kept resident in SBUF, then multiplied pointwise,
  followed by the mirrored inverse factorization (only the first 4096
  output samples are produced).

Host side packs x into the per-core DMA-friendly layout
  U[p, n1, h', n2] = x[2p(+1), 128*n1+n2, h0+h']   (bf16)
and unpacks the per-core output O[b, n1, h', n2] -> out[b, 128*n1+n2, h].
"""

import sys

sys.path.insert(0, "/opt/trn_rl_repo")

import numpy as np
import ml_dtypes

B, L, H = 4, 4096, 1024
NCORES = 8
HSH = H // NCORES  # 128 channels per core
N = 2 * L  # 8192
N1, N2 = 64, 128  # N = N1 * N2
S = 16  # sequences (h' channels) per tile
BF = ml_dtypes.bfloat16

_cache = {}


def _consts():
    n1 = np.arange(32)[:, None]
    k1 = np.arange(64)[None, :]
    W1 = np.exp(-2j * np.pi * (n1 * k1) / 64.0)  # [32,64] lhsT for step1
    n2 = np.arange(128)[:, None]
    k2 = np.arange(128)[None, :]
    W2 = np.exp(-2j * np.pi * (n2 * k2) / 128.0)  # [128,128] lhsT step2
    WA = np.exp(2j * np.pi * (n2 * k2) / 128.0)  # [128,128] lhsT invA (sym)
    k1c = np.arange(64)[:, None]
    m1 = np.arange(32)[None, :]
    WC = np.exp(2j * np.pi * (k1c * m1) / 64.0)  # [64,32] lhsT invC
    # forward twiddle T[k1,n2] = exp(-2pi i k1*n2/8192), tiled S times
    Tf = np.exp(-2j * np.pi * np.outer(np.arange(64), np.arange(128)) / 8192.0)
    # inverse twiddle Ti[n2,k1] = exp(+2pi i n2*k1/8192), tiled S times
    Ti = np.exp(2j * np.pi * np.outer(np.arange(128), np.arange(64)) / 8192.0)

    def b(a):
        return np.ascontiguousarray(a).astype(BF)

    c = {}
    for nm, M in (("s1", W1), ("s2", W2), ("sa", WA), ("sc", WC)):
        c[nm + "re"] = b(M.real)
        c[nm + "im"] = b(M.imag)
        c[nm + "imneg"] = b(-M.imag)
    for nm, M in (("s1f", W1 / N),):
        c[nm + "re"] = b(M.real)
        c[nm + "im"] = b(M.imag)
    c["tfre"] = np.ascontiguousarray(np.tile(Tf.real, (1, S))).astype(np.float32)
    c["tfim"] = np.ascontiguousarray(np.tile(Tf.imag, (1, S))).astype(np.float32)
    c["tire"] = np.ascontiguousarray(np.tile(Ti.real, (1, S))).astype(np.float32)
    c["tiim"] = np.ascontiguousarray(np.tile(Ti.imag, (1, S))).astype(np.float32)
    return c


def _build():
    import concourse.mybir as mybir
    import concourse.tile as tile
    from concourse import bacc
    from concourse.masks import make_identity

    bf16 = mybir.dt.bfloat16
    f32 = mybir.dt.float32

    nc = bacc.Bacc("TRN2", target_bir_lowering=False, debug=False, num_devices=NCORES)

    ure_d = nc.dram_tensor("ure", [2, 32, HSH, 128], bf16, kind="ExternalInput").ap()
    uim_d = nc.dram_tensor("uim", [2, 32, HSH, 128], bf16, kind="ExternalInput").ap()
    fz_d = nc.dram_tensor("fz", [32, HSH, 128], bf16, kind="ExternalInput").ap()
    cst = {}
    for nm in ("s1re", "s1im", "s1imneg", "s1fre", "s1fim"):
        cst[nm] = nc.dram_tensor(nm, [32, 64], bf16, kind="ExternalInput").ap()
    for nm in ("s2re", "s2im", "s2imneg", "sare", "saim", "saimneg"):
        cst[nm] = nc.dram_tensor(nm, [128, 128], bf16, kind="ExternalInput").ap()
    for nm in ("scre", "scim", "scimneg"):
        cst[nm] = nc.dram_tensor(nm, [64, 32], bf16, kind="ExternalInput").ap()
    for nm in ("tfre", "tfim"):
        cst[nm] = nc.dram_tensor(nm, [64, 128 * S], f32, kind="ExternalInput").ap()
    for nm in ("tire", "tiim"):
        cst[nm] = nc.dram_tensor(nm, [128, 64 * S], f32, kind="ExternalInput").ap()
    oc_d = nc.dram_tensor("oc", [4, 32, HSH, 128], f32, kind="ExternalOutput").ap()

    NT = HSH // S  # tiles per (pair / filter) pass

    with tile.TileContext(nc) as tc:
        with (
            tc.tile_pool(name="consts", bufs=1) as cpool,
            tc.tile_pool(name="kf", bufs=1) as kfpool,
            tc.tile_pool(name="work", bufs=2) as wp,
            tc.tile_pool(name="psmm", bufs=4, space="PSUM") as pmm,
            tc.tile_pool(name="pstr", bufs=4, space="PSUM") as ptr,
            # NOTE: pmm/ptr each use ONE shared tag so the static PSUM
            # footprint is 4+4 banks (PSUM total is 8 x 2KB banks).
        ):
            # resident constants
            sb = {}
            for nm, ap in cst.items():
                t = cpool.tile(list(ap.shape), ap.dtype, name=f"c_{nm}")
                nc.sync.dma_start(t[:], ap[:])
                sb[nm] = t
            id64 = cpool.tile([64, 64], bf16, name="id64")
            make_identity(nc, id64[:])
            id128 = cpool.tile([128, 128], bf16, name="id128")
            make_identity(nc, id128[:])

            # resident filter FFT: [k2=128, h'(128) x k1(64)] re/im
            kfre = kfpool.tile([128, HSH * 64], bf16, name="kfre")
            kfim = kfpool.tile([128, HSH * 64], bf16, name="kfim")

            def fwd_tile(are, aim, s1re_t, s1im_t, s1imneg_t, xre_out, xim_out, ocol0):
                """Forward FFT of S sequences.

                are/aim: [32, S*128] bf16 SBUF (aim None => real input).
                Writes X re/im into xre_out/xim_out at column ocol0
                (layout [k2=128, (s,k1)] => S*64 cols).
                """
                cre = wp.tile([64, S * 128], bf16, tag="cre")
                cim = wp.tile([64, S * 128], bf16, tag="cim")
                for c in range(S * 128 // 512):
                    sl = slice(c * 512, (c + 1) * 512)
                    bre = pmm.tile([64, 512], f32, tag="mm")
                    bim = pmm.tile([64, 512], f32, tag="mm")
                    if aim is None:
                        nc.tensor.matmul(bre[:], s1re_t[:], are[:, sl], start=True, stop=True)
                        nc.tensor.matmul(bim[:], s1im_t[:], are[:, sl], start=True, stop=True)
                    else:
                        nc.tensor.matmul(bre[:], s1re_t[:], are[:, sl], start=True, stop=False)
                        nc.tensor.matmul(bre[:], s1imneg_t[:], aim[:, sl], start=False, stop=True)
                        nc.tensor.matmul(bim[:], s1im_t[:], are[:, sl], start=True, stop=False)
                        nc.tensor.matmul(bim[:], s1re_t[:], aim[:, sl], start=False, stop=True)
                    # twiddle: C = B * Tf   (fp32 psum x fp32 sbuf -> bf16)
                    t1 = wp.tile([64, 512], f32, tag="tw1")
                    t2 = wp.tile([64, 512], f32, tag="tw2")
                    nc.vector.tensor_mul(t1[:], bre[:], sb["tfre"][:, sl])
                    nc.vector.tensor_mul(t2[:], bim[:], sb["tfim"][:, sl])
                    nc.vector.tensor_sub(cre[:, sl], t1[:], t2[:])
                    nc.vector.tensor_mul(t1[:], bre[:], sb["tfim"][:, sl])
                    nc.vector.tensor_mul(t2[:], bim[:], sb["tfre"][:, sl])
                    nc.vector.tensor_add(cim[:, sl], t1[:], t2[:])
                # transpose C per sequence -> ct [n2=128, (s,k1)]
                ctre = wp.tile([128, S * 64], bf16, tag="ctre")
                ctim = wp.tile([128, S * 64], bf16, tag="ctim")
                for half, (csb, ctsb) in enumerate(((cre, ctre), (cim, ctim))):
                    for g in range(S // 4):  # groups of 4 seqs per psum bank
                        ptile = ptr.tile([128, 256], bf16, tag="tr")
                        for j in range(4):
                            s_ = g * 4 + j
                            nc.tensor.transpose(
                                ptile[:, j * 64 : (j + 1) * 64],
                                csb[:, s_ * 128 : (s_ + 1) * 128],
                                id64[:],
                            )
                        nc.any.tensor_copy(
                            out=ctsb[:, g * 256 : (g + 1) * 256], in_=ptile[:]
                        )
                # step2: X = W128^T @ C^T
                for c in range(S * 64 // 512):
                    sl = slice(c * 512, (c + 1) * 512)
                    osl = slice(ocol0 + c * 512, ocol0 + (c + 1) * 512)
                    xre = pmm.tile([128, 512], f32, tag="mm")
                    xim = pmm.tile([128, 512], f32, tag="mm")
                    nc.tensor.matmul(xre[:], sb["s2re"][:], ctre[:, sl], start=True, stop=False)
                    nc.tensor.matmul(xre[:], sb["s2imneg"][:], ctim[:, sl], start=False, stop=True)
                    nc.tensor.matmul(xim[:], sb["s2im"][:], ctre[:, sl], start=True, stop=False)
                    nc.tensor.matmul(xim[:], sb["s2re"][:], ctim[:, sl], start=False, stop=True)
                    nc.any.tensor_copy(out=xre_out[:, osl], in_=xre[:])
                    nc.any.tensor_copy(out=xim_out[:, osl], in_=xim[:])

            # ---- filter pass: real input, scaled stationaries ----
            for it in range(NT):
                fa = wp.tile([32, S * 128], bf16, tag="fa")
                nc.sync.dma_start(
                    fa[:],
                    fz_d[:, it * S : (it + 1) * S, :].rearrange("a b c -> a (b c)"),
                )
                fwd_tile(fa, None, sb["s1fre"], sb["s1fim"], None, kfre, kfim,
                         it * S * 64)

            # ---- data passes ----
            for p in range(2):
                for it in range(NT):
                    are = wp.tile([32, S * 128], bf16, tag="are")
                    aim = wp.tile([32, S * 128], bf16, tag="aim")
                    nc.sync.dma_start(
                        are[:],
                        ure_d[p, :, it * S : (it + 1) * S, :].rearrange(
                            "a b c -> a (b c)"
                        ),
                    )
                    nc.sync.dma_start(
                        aim[:],
                        uim_d[p, :, it * S : (it + 1) * S, :].rearrange(
                            "a b c -> a (b c)"
                        ),
                    )
                    xre_sb = wp.tile([128, S * 64], bf16, tag="xre")
                    xim_sb = wp.tile([128, S * 64], bf16, tag="xim")
                    fwd_tile(are, aim, sb["s1re"], sb["s1im"], sb["s1imneg"],
                             xre_sb, xim_sb, 0)

                    # pointwise P = X * Kf  (bf16)
                    pre = wp.tile([128, S * 64], bf16, tag="pre")
                    pim = wp.tile([128, S * 64], bf16, tag="pim")
                    kcol = slice(it * S * 64, (it + 1) * S * 64)
                    t1 = wp.tile([128, S * 64], bf16, tag="pw1")
                    t2 = wp.tile([128, S * 64], bf16, tag="pw2")
                    nc.vector.tensor_mul(t1[:], xre_sb[:], kfre[:, kcol])
                    nc.vector.tensor_mul(t2[:], xim_sb[:], kfim[:, kcol])
                    nc.vector.tensor_sub(pre[:], t1[:], t2[:])
                    nc.vector.tensor_mul(t1[:], xre_sb[:], kfim[:, kcol])
                    nc.vector.tensor_mul(t2[:], xim_sb[:], kfre[:, kcol])
                    nc.vector.tensor_add(pim[:], t1[:], t2[:])

                    # inverse stepA: Q = WA^T @ P ; twiddle -> R
                    rre = wp.tile([128, S * 64], bf16, tag="rre")
                    rim = wp.tile([128, S * 64], bf16, tag="rim")
                    for c in range(S * 64 // 512):
                        sl = slice(c * 512, (c + 1) * 512)
                        qre = pmm.tile([128, 512], f32, tag="mm")
                        qim = pmm.tile([128, 512], f32, tag="mm")
                        nc.tensor.matmul(qre[:], sb["sare"][:], pre[:, sl], start=True, stop=False)
                        nc.tensor.matmul(qre[:], sb["saimneg"][:], pim[:, sl], start=False, stop=True)
                        nc.tensor.matmul(qim[:], sb["saim"][:], pre[:, sl], start=True, stop=False)
                        nc.tensor.matmul(qim[:], sb["sare"][:], pim[:, sl], start=False, stop=True)
                        t3 = wp.tile([128, 512], f32, tag="it1")
                        t4 = wp.tile([128, 512], f32, tag="it2")
                        nc.vector.tensor_mul(t3[:], qre[:], sb["tire"][:, sl])
                        nc.vector.tensor_mul(t4[:], qim[:], sb["tiim"][:, sl])
                        nc.vector.tensor_sub(rre[:, sl], t3[:], t4[:])
                        nc.vector.tensor_mul(t3[:], qre[:], sb["tiim"][:, sl])
                        nc.vector.tensor_mul(t4[:], qim[:], sb["tire"][:, sl])
                        nc.vector.tensor_add(rim[:, sl], t3[:], t4[:])

                    # transpose R per sequence -> rt [k1=64, (s,n2=128)]
                    rtre = wp.tile([64, S * 128], bf16, tag="rtre")
                    rtim = wp.tile([64, S * 128], bf16, tag="rtim")
                    for half, (rsb, rtsb) in enumerate(((rre, rtre), (rim, rtim))):
                        for g in range(S // 4):
                            ptile = ptr.tile([64, 512], bf16, tag="tr")
                            for j in range(4):
                                s_ = g * 4 + j
                                nc.tensor.transpose(
                                    ptile[:, j * 128 : (j + 1) * 128],
                                    rsb[:, s_ * 64 : (s_ + 1) * 64],
                                    id128[:],
                                )
                            nc.any.tensor_copy(
                                out=rtsb[:, g * 512 : (g + 1) * 512], in_=ptile[:]
                            )

                    # inverse stepC: Y = WC^T @ R^T ; out = Y (re->2p, im->2p+1)
                    for c in range(S * 128 // 512):
                        sl = slice(c * 512, (c + 1) * 512)
                        yre = pmm.tile([32, 512], f32, tag="mm")
                        yim = pmm.tile([32, 512], f32, tag="mm")
                        nc.tensor.matmul(yre[:], sb["scre"][:], rtre[:, sl], start=True, stop=False)
                        nc.tensor.matmul(yre[:], sb["scimneg"][:], rtim[:, sl], start=False, stop=True)
                        nc.tensor.matmul(yim[:], sb["scim"][:], rtre[:, sl], start=True, stop=False)
                        nc.tensor.matmul(yim[:], sb["scre"][:], rtim[:, sl], start=False, stop=True)
                        ore = wp.tile([32, 512], f32, tag="ore")
                        oim = wp.tile([32, 512], f32, tag="oim")
                        nc.any.tensor_copy(out=ore[:], in_=yre[:])
                        nc.any.tensor_copy(out=oim[:], in_=yim[:])
                        # cols c*512.. = seqs s in [c*4,(c+1)*4), n2 full
                        hsl = slice(it * S + c * 4, it * S + (c + 1) * 4)
                        nc.sync.dma_start(
                            oc_d[2 * p, :, hsl, :].rearrange("a b c -> a (b c)"),
                            ore[:],
                        )
                        nc.sync.dma_start(
                            oc_d[2 * p + 1, :, hsl, :].rearrange("a b c -> a (b c)"),
                            oim[:],
                        )

    nc.compile()
    return nc


def _prep_inputs(x, filt):
    consts = _consts()
    filt2 = filt.copy()
    filt2[:, 0] += 1.0  # fold residual: conv with (filt + delta) = y + u
    # U_all[b, n1, h, n2] = x[b, 128*n1+n2, h]
    u_all = np.ascontiguousarray(
        x.reshape(B, 32, 128, H).transpose(0, 1, 3, 2)
    ).astype(BF)  # [4,32,1024,128]
    f_all = np.ascontiguousarray(
        filt2.reshape(H, 32, 128).transpose(1, 0, 2)
    ).astype(BF)  # [32,1024,128]
    in_maps = []
    for c in range(NCORES):
        h0 = c * HSH
        m = dict(consts)
        usl = u_all[:, :, h0 : h0 + HSH, :]  # [4,32,128,128]
        m["ure"] = np.ascontiguousarray(usl[0::2])  # batches 0,2
        m["uim"] = np.ascontiguousarray(usl[1::2])  # batches 1,3
        m["fz"] = np.ascontiguousarray(f_all[:, h0 : h0 + HSH, :])
        in_maps.append(m)
    return in_maps


def _postprocess(results):
    out = np.empty((B, L, H), np.float32)
    ov = out.reshape(B, 32, 128, H)
    for c in range(NCORES):
        oc = results[c]["oc"]  # [4,32,128,128] = (b, n1, h', n2)
        ov[:, :, :, c * HSH : (c + 1) * HSH] = oc.transpose(0, 1, 3, 2)
    return out


def _get_nc():
    if "nc" not in _cache:
        _cache["nc"] = _build()
    return _cache["nc"]


def _get_exec():
    """Jitted shard_map executable over 8 cores (mirrors
    bass2jax.run_bass_via_pjrt, but cached so repeat calls don't
    re-trace, and without output donation so input device buffers can
    be reused for timing)."""
    if "exec" in _cache:
        return _cache["exec"]
    import jax
    from jax.sharding import Mesh, PartitionSpec
    from jax.experimental.shard_map import shard_map
    import concourse.mybir as mybir
    from concourse import bass2jax

    nc = _get_nc()
    bass2jax.install_neuronx_cc_hook()
    assert nc.dbg_addr is None
    pname = nc.partition_id_tensor.name if nc.partition_id_tensor else None
    in_names, out_names, out_avals, zero_outs = [], [], [], []
    for alloc in nc.m.functions[0].allocations:
        if not isinstance(alloc, mybir.MemoryLocationSet):
            continue
        name = alloc.memorylocations[0].name
        if alloc.kind == "ExternalInput":
            if name != pname:
                in_names.append(name)
        elif alloc.kind == "ExternalOutput":
            out_names.append(name)
            shape = tuple(alloc.tensor_shape)
            dtype = mybir.dt.np(alloc.dtype)
            out_avals.append(jax.core.ShapedArray(shape, dtype))
            zero_outs.append(np.zeros((NCORES * shape[0], *shape[1:]), dtype))
    all_names = in_names + out_names
    if pname is not None:
        all_names = all_names + [pname]

    def _body(*args):
        operands = list(args)
        if pname is not None:
            operands.append(bass2jax.partition_id_tensor())
        outs = bass2jax._bass_exec_p.bind(
            *operands,
            out_avals=tuple(out_avals),
            in_names=tuple(all_names),
            out_names=tuple(out_names),
            lowering_input_output_aliases=(),
            sim_require_finite=True,
            sim_require_nnan=True,
            nc=nc,
        )
        return tuple(outs)

    mesh = Mesh(np.asarray(jax.devices()[:NCORES]), ("core",))
    nin = len(in_names) + len(out_names)
    sharded = jax.jit(
        shard_map(
            _body,
            mesh=mesh,
            in_specs=(PartitionSpec("core"),) * nin,
            out_specs=(PartitionSpec("core"),) * len(out_names),
            check_rep=False,
        ),
        keep_unused=True,
    )
    _cache["exec"] = (sharded, in_names, out_names, mesh, zero_outs)
    return _cache["exec"]


def _concat_inputs(in_maps, in_names):
    return [
        np.concatenate([in_maps[c][nm] for c in range(NCORES)], axis=0)
        for nm in in_names
    ]


def kernel(x: np.ndarray, filt: np.ndarray) -> np.ndarray:
    x = np.asarray(x, dtype=np.float32)
    filt = np.asarray(filt, dtype=np.float32)
    sharded, in_names, out_names, mesh, zero_outs = _get_exec()
    in_maps = _prep_inputs(x, filt)
    outs = sharded(*_concat_inputs(in_maps, in_names), *zero_outs)
    oc_all = np.asarray(outs[0]).reshape(NCORES, 4, 32, HSH, 128)
    out = np.empty((B, L, H), np.float32)
    ov = out.reshape(B, 32, 128, H)
    for c in range(NCORES):
        ov[:, :, :, c * HSH : (c + 1) * HSH] = oc_all[c].transpose(0, 1, 3, 2)
    return out


def measure_hw_ns(x, filt, iters=10):
    """Marginal per-execution device time: dispatch (1 + iters) NEFF
    executions with all inputs resident on device, single sync at the
    end of each batch; the difference isolates device execution from
    tunnel transfer/dispatch latency."""
    import time
    import jax
    from jax.sharding import NamedSharding, PartitionSpec

    x = np.asarray(x, dtype=np.float32)
    filt = np.asarray(filt, dtype=np.float32)
    sharded, in_names, out_names, mesh, zero_outs = _get_exec()
    sh = NamedSharding(mesh, PartitionSpec("core"))
    in_maps = _prep_inputs(x, filt)
    dev_args = [
        jax.device_put(a, sh)
        for a in (*_concat_inputs(in_maps, in_names), *zero_outs)
    ]
    jax.block_until_ready(dev_args)

    def run_n(n):
        t0 = time.perf_counter()
        res = None
        for _ in range(n):
            res = sharded(*dev_args)
        jax.block_until_ready(res)
        return time.perf_counter() - t0

    run_n(2)  # warmup
    t1 = min(run_n(1) for _ in range(3))
    tn = min(run_n(1 + iters) for _ in range(3))
    return max(1, int((tn - t1) / iters * 1e9))
